# revision 25
# baseline (speedup 1.0000x reference)
"""Trainium2 Bass kernel for nn_BertFreezeSegmentor (BiLSTM + stack-decoder).

Restructuring (validated vs the reference in numpy, resid_var ~9e-6 with
bf16 weights):
  - Gold actions are in {0,1}, so the decoder "stacks" collapse into
    conditional carries: the subword stack read is the previous step's
    (h1,c1) when gold==0 and zeros when gold==1; the word stack read is a
    hold/update carry (updates when gold==1).
  - All x-projections (Wih matmuls, classifier) are hoisted out of the four
    recurrences (fwd scan, bwd scan, subword chain, word chain) into big
    GEMMs; each recurrence step only needs its h @ Whh.T matmul.
  - Recurrences run weights-stationary on the PE with gates in transposed
    layout [4H, B] so elementwise work and the next step's moving operand
    need no transposes.
  - bf16 weights/h, fp32 PSUM accumulation and fp32 carries.

Sharding: pure data parallelism, 8 examples per core on 8 cores. All
per-core differences (batch slice, masks) are input data, so one SPMD
program serves every core.
"""

import numpy as np
import ml_dtypes

import concourse.bass as bass
import concourse.tile as tile
from concourse import bacc, mybir
from concourse.bass_utils import run_bass_kernel_spmd

BF16 = ml_dtypes.bfloat16
DT_BF = mybir.dt.bfloat16
DT_F32 = mybir.dt.float32
AF = mybir.ActivationFunctionType

FULL = dict(S=256, B=8, H=768, NCORES=8)


# --------------------------------------------------------------------------
# program builder
# --------------------------------------------------------------------------

def build_program(S, B, H, num_devices=8, unroll=8, phases="ABCDEFG"):
    CH = H // 128          # h chunks (6)
    GM = 4 * H // 128      # gate m-tiles (24)
    C2 = 2 * H // 128      # lstm_out / [h1;c1] chunks (12)
    NC = S * B             # (t,b) columns (2048)
    NB = min(512, NC)      # GEMM N-block
    NBLK = NC // NB
    assert S % unroll == 0 and H % 128 == 0 and NC % NB == 0 and NB % B == 0

    nc = bacc.Bacc("TRN2", target_bir_lowering=False, debug=False,
                   enable_asserts=False, num_devices=num_devices)

    def inp(name, shape, dt):
        return nc.dram_tensor(name, shape, dt, kind="ExternalInput").ap()

    def scratch(name, shape, dt):
        return nc.dram_tensor(name, shape, dt, kind="Internal").ap()

    def outp(name, shape, dt):
        return nc.dram_tensor(name, shape, dt, kind="ExternalOutput").ap()

    # ---- inputs ----
    xT = inp("xT", [128, CH, NC], DT_BF)
    xTr = inp("xTr", [128, CH, NC], DT_BF)
    wih_f = inp("wih_f", [128, CH, 4 * H], DT_BF)
    whh_f = inp("whh_f", [128, CH, 4 * H], DT_BF)
    wih_b = inp("wih_b", [128, CH, 4 * H], DT_BF)
    whh_b = inp("whh_b", [128, CH, 4 * H], DT_BF)
    bias_f = inp("bias_f", [1, 4 * H], DT_BF)
    bias_b = inp("bias_b", [1, 4 * H], DT_BF)
    swih = inp("swih", [128, C2, 4 * H], DT_BF)
    swhh = inp("swhh", [128, CH, 4 * H], DT_BF)
    sbias = inp("sbias", [1, 4 * H], DT_BF)
    wwih = inp("wwih", [128, C2, 4 * H], DT_BF)
    wwhh = inp("wwhh", [128, CH, 4 * H], DT_BF)
    wbias = inp("wbias", [1, 4 * H], DT_BF)
    cls1T = inp("cls1T", [128, CH, 2], DT_BF)
    cls2T = inp("cls2T", [128, C2, 2], DT_BF)
    keep6 = inp("keep6", [128, CH, NC], DT_BF)
    wsel6 = inp("wsel6", [128, CH, NC], DT_BF)

    # ---- DRAM scratch ----
    XFT = scratch("XFT", [128, GM, NC], DT_BF)
    XBT = scratch("XBT", [128, GM, NC], DT_BF)
    SDT = scratch("SDT", [128, GM, NC], DT_BF)
    WIT = scratch("WIT", [128, GM, NC], DT_BF)

    # ---- outputs ----
    cx_t = outp("cx_t", [2, NC], DT_F32)
    wcls_t = outp("wcls_t", [2, NC], DT_F32)

    with tile.TileContext(nc) as tc:

        _dma_rr = [0]

        def dma_eng():
            _dma_rr[0] += 1
            return nc.sync if _dma_rr[0] % 2 else nc.gpsimd

        def load_w(pool, src, tag):
            t = pool.tile(list(src.shape), src.dtype, tag=tag)
            if len(src.shape) == 3 and src.shape[1] > 1:
                # per-chunk loads alternate queues and unblock consumers early
                for k in range(src.shape[1]):
                    dma_eng().dma_start(t[:, k, :], src[:, k, :])
            else:
                dma_eng().dma_start(t[:], src[:])
            return t

        # ==============================================================
        # Phase A: XF / XB GEMMs
        # ==============================================================
        if "A" in phases:
         with tc.tile_pool(name="wA", bufs=1) as wpool, \
             tc.tile_pool(name="gA", bufs=3) as pool, \
             tc.tile_pool(name="gA_ps", bufs=2, space=bass.MemorySpace.PSUM) as psp:
            ones = wpool.tile([1, NB], DT_BF, tag="ones")
            nc.vector.memset(ones[:], 1.0)
            xT_sb = load_w(wpool, xT, "xT_sb")
            xTr_sb = load_w(wpool, xTr, "xTr_sb")
            wf_sb = load_w(wpool, wih_f, "wf_sb")
            wb_sb = load_w(wpool, wih_b, "wb_sb")
            bf_sb = load_w(wpool, bias_f, "bf_sb")
            bb_sb = load_w(wpool, bias_b, "bb_sb")
            for (wih, bia, mv, dst) in ((wf_sb, bf_sb, xT_sb, XFT),
                                        (wb_sb, bb_sb, xTr_sb, XBT)):
                for m in range(GM):
                    for nb in range(NBLK):
                        ps = psp.tile([128, NB], DT_F32, tag="ps")
                        for k in range(CH):
                            nc.tensor.matmul(
                                ps[:], wih[:, k, bass.ts(m, 128)],
                                mv[:, k, bass.ts(nb, NB)],
                                start=(k == 0), stop=False)
                        nc.tensor.matmul(
                            ps[:], bia[:, bass.ts(m, 128)], ones[:],
                            start=False, stop=True)
                        ot = pool.tile([128, NB], DT_BF, tag="gout")
                        nc.vector.tensor_copy(ot[:], ps[:])
                        dma_eng().dma_start(dst[:, m, bass.ts(nb, NB)], ot[:])

        # ==============================================================
        # Phase B: scans (+ reversal)   Phase C: SD / CX GEMMs
        # ==============================================================
        with tc.tile_pool(name="histA", bufs=1) as histA:
            lstm_fT = histA.tile([128, CH, S + 1, B], DT_BF, tag="lstm_fT")
            lstm_bT = histA.tile([128, CH, S + 1, B], DT_BF, tag="lstm_bT")
            lstm_bRT = histA.tile([128, CH, S, B], DT_BF, tag="lstm_bRT")

            def dual_scan(whhf_dram, whhb_dram):
                # fwd and bwd scans interleaved in one loop: each direction's
                # serial elementwise tail hides under the other's matmuls.
                with tc.tile_pool(name="w_scan", bufs=1) as wp, \
                     tc.tile_pool(name="scan", bufs=3) as sp, \
                     tc.tile_pool(name="scan_ps", bufs=2,
                                  space=bass.MemorySpace.PSUM) as pp:
                    dirs = []
                    for dd, (whh_dram, src, dstT) in enumerate(
                            ((whhf_dram, XFT, lstm_fT),
                             (whhb_dram, XBT, lstm_bT))):
                        whh = load_w(wp, whh_dram, f"whh_sb{dd}")
                        c0 = wp.tile([128, CH, B], DT_F32, tag=f"c0{dd}")
                        c1 = wp.tile([128, CH, B], DT_F32, tag=f"c1{dd}")
                        h0 = wp.tile([128, CH, B], DT_BF, tag=f"h0{dd}")
                        h1 = wp.tile([128, CH, B], DT_BF, tag=f"h1{dd}")
                        nc.vector.memset(c0[:], 0.0)
                        nc.vector.memset(h0[:], 0.0)
                        nc.vector.memset(dstT[:, :, 0, :], 0.0)
                        dirs.append(dict(whh=whh, src=src, dstT=dstT,
                                         cc=[c0, c1], hh=[h0, h1], dd=dd))

                    def step(D, xf, i, t):
                        cc, hh, dd = D["cc"], D["hh"], D["dd"]
                        cprev, cnew = cc[i % 2], cc[(i + 1) % 2]
                        hprev, hnext = hh[i % 2], hh[(i + 1) % 2]
                        ps = pp.tile([128, GM, B], DT_F32, tag=f"g{dd}")
                        for m in range(GM):
                            for k in range(CH):
                                nc.tensor.matmul(
                                    ps[:, m, :],
                                    D["whh"][:, k, bass.ts(m, 128)],
                                    hprev[:, k, :],
                                    start=(k == 0), stop=(k == CH - 1))
                        g = sp.tile([128, GM, B], DT_F32, tag=f"gs{dd}")
                        nc.vector.tensor_add(
                            g[:], ps[:], xf[:, :, i * B:(i + 1) * B])
                        sif = sp.tile([128, 2 * CH, B], DT_F32, tag=f"sif{dd}")
                        nc.scalar.activation(sif[:], g[:, 0:2 * CH, :],
                                             AF.Sigmoid)
                        tg = sp.tile([128, CH, B], DT_F32, tag=f"tg{dd}")
                        nc.scalar.activation(tg[:], g[:, 2 * CH:3 * CH, :],
                                             AF.Tanh)
                        so = sp.tile([128, CH, B], DT_F32, tag=f"so{dd}")
                        nc.scalar.activation(so[:], g[:, 3 * CH:4 * CH, :],
                                             AF.Sigmoid)
                        t1 = sp.tile([128, CH, B], DT_F32, tag=f"t1{dd}")
                        nc.vector.tensor_mul(t1[:], sif[:, CH:2 * CH, :],
                                             cprev[:])
                        t2 = sp.tile([128, CH, B], DT_F32, tag=f"t2{dd}")
                        nc.vector.tensor_mul(t2[:], sif[:, 0:CH, :], tg[:])
                        nc.vector.tensor_add(cnew[:], t1[:], t2[:])
                        th = sp.tile([128, CH, B], DT_F32, tag=f"th{dd}")
                        nc.scalar.activation(th[:], cnew[:], AF.Tanh)
                        nc.vector.tensor_mul(hnext[:], so[:], th[:])
                        nc.vector.tensor_copy(
                            D["dstT"][:, :, bass.ds(t + 1, 1), :], hnext[:])

                    def blk(iv0, cnt):
                        xfs = []
                        for D in dirs:
                            xf = sp.tile([128, GM, unroll * B], DT_BF,
                                         tag=f"xf{D['dd']}")
                            nc.sync.dma_start(
                                xf[:, :, 0:cnt * B],
                                D["src"][:, :, bass.ds(iv0 * B, cnt * B)])
                            xfs.append(xf)
                        for i in range(cnt):
                            step(dirs[0], xfs[0], i, iv0 + i)
                            step(dirs[1], xfs[1], i, iv0 + i)

                    tc.For_i_unrolled_general(0, S, 1, blk, max_unroll=unroll)

            if "B" in phases:
                dual_scan(whh_f, whh_b)
                for t in range(S):
                    nc.vector.tensor_copy(
                        lstm_bRT[:, :, t, :], lstm_bT[:, :, S - t, :])

            def lstm_mv(k, nb):
                # moving operand [128, NB]: lstm_out chunk k at time t=col//B
                # (flat 1-D free slice - 2-D free APs stream ~10x slower)
                t0 = (nb * NB) // B
                if k < CH:
                    fl = lstm_fT[:, k].rearrange("p a b -> p (a b)")
                    return fl[:, (t0 + 1) * B:(t0 + 1) * B + NB]
                fl = lstm_bRT[:, k - CH].rearrange("p a b -> p (a b)")
                return fl[:, t0 * B:t0 * B + NB]

            if "C" in phases:
             with tc.tile_pool(name="wC", bufs=1) as wp, \
                 tc.tile_pool(name="gC", bufs=3) as pool, \
                 tc.tile_pool(name="gC_ps", bufs=2,
                              space=bass.MemorySpace.PSUM) as psp:
                ones = wp.tile([1, NB], DT_BF, tag="onesC")
                nc.vector.memset(ones[:], 1.0)
                swih_sb = load_w(wp, swih, "swih_sb")
                sb_sb = load_w(wp, sbias, "sb_sb")
                c2_sb = load_w(wp, cls2T, "c2_sb")
                for m in range(GM):
                    for nb in range(NBLK):
                        ps = psp.tile([128, NB], DT_F32, tag="psC")
                        for k in range(C2):
                            nc.tensor.matmul(
                                ps[:], swih_sb[:, k, bass.ts(m, 128)],
                                lstm_mv(k, nb),
                                start=(k == 0), stop=False)
                        nc.tensor.matmul(
                            ps[:], sb_sb[:, bass.ts(m, 128)], ones[:],
                            start=False, stop=True)
                        ot = pool.tile([128, NB], DT_BF, tag="goutC")
                        nc.vector.tensor_copy(ot[:], ps[:])
                        dma_eng().dma_start(SDT[:, m, bass.ts(nb, NB)], ot[:])
                for nb in range(NBLK):
                    ps = psp.tile([2, NB], DT_F32, tag="psCX")
                    for k in range(C2):
                        nc.tensor.matmul(
                            ps[:], c2_sb[:, k, :], lstm_mv(k, nb),
                            start=(k == 0), stop=(k == C2 - 1))
                    ot = pool.tile([2, NB], DT_F32, tag="cxout")
                    nc.vector.tensor_copy(ot[:], ps[:])
                    nc.sync.dma_start(cx_t[:, bass.ts(nb, NB)], ot[:])

        # ==============================================================
        # decode recurrences
        # ==============================================================
        def dec_chain(whh_dram, srcT, outH, outC, sel_mask, is_word):
            with tc.tile_pool(name="w_dch", bufs=1) as wp, \
                 tc.tile_pool(name="dch", bufs=3) as sp, \
                 tc.tile_pool(name="dch_ps", bufs=2,
                              space=bass.MemorySpace.PSUM) as pp:
                whh = load_w(wp, whh_dram, "whh_dch")
                hA = wp.tile([128, CH, B], DT_BF, tag="hA")
                hB = wp.tile([128, CH, B], DT_BF, tag="hB")
                cA = wp.tile([128, CH, B], DT_F32, tag="cA")
                cB = wp.tile([128, CH, B], DT_F32, tag="cB")
                nc.vector.memset(hA[:], 0.0)
                nc.vector.memset(cA[:], 0.0)
                hh, ccy = [hA, hB], [cA, cB]

                def blk(iv0, cnt):
                    xf = sp.tile([128, GM, unroll * B], DT_BF, tag="xfD")
                    nc.sync.dma_start(
                        xf[:, :, 0:cnt * B],
                        srcT[:, :, bass.ds(iv0 * B, cnt * B)])
                    msk = sp.tile([128, CH, unroll * B], DT_BF, tag="mskD")
                    nc.sync.dma_start(
                        msk[:, :, 0:cnt * B],
                        sel_mask[:, :, bass.ds(iv0 * B, cnt * B)])
                    for i in range(cnt):
                        hprev, hnext = hh[i % 2], hh[(i + 1) % 2]
                        cprev, cnext = ccy[i % 2], ccy[(i + 1) % 2]
                        ps = pp.tile([128, GM, B], DT_F32, tag="gD")
                        for m in range(GM):
                            for k in range(CH):
                                nc.tensor.matmul(
                                    ps[:, m, :],
                                    whh[:, k, bass.ts(m, 128)],
                                    hprev[:, k, :],
                                    start=(k == 0), stop=(k == CH - 1))
                        g = sp.tile([128, GM, B], DT_F32, tag="gsD")
                        nc.vector.tensor_add(
                            g[:], ps[:], xf[:, :, i * B:(i + 1) * B])
                        sif = sp.tile([128, 2 * CH, B], DT_F32, tag="sifD")
                        nc.scalar.activation(sif[:], g[:, 0:2 * CH, :],
                                             AF.Sigmoid)
                        tg = sp.tile([128, CH, B], DT_F32, tag="tgD")
                        nc.scalar.activation(tg[:], g[:, 2 * CH:3 * CH, :],
                                             AF.Tanh)
                        so = sp.tile([128, CH, B], DT_F32, tag="soD")
                        nc.scalar.activation(so[:], g[:, 3 * CH:4 * CH, :],
                                             AF.Sigmoid)
                        t1 = sp.tile([128, CH, B], DT_F32, tag="t1D")
                        nc.vector.tensor_mul(t1[:], sif[:, CH:2 * CH, :],
                                             cprev[:])
                        t2 = sp.tile([128, CH, B], DT_F32, tag="t2D")
                        nc.vector.tensor_mul(t2[:], sif[:, 0:CH, :], tg[:])
                        cf = sp.tile([128, CH, B], DT_F32, tag="cfD")
                        nc.vector.tensor_add(cf[:], t1[:], t2[:])
                        th = sp.tile([128, CH, B], DT_F32, tag="thD")
                        nc.scalar.activation(th[:], cf[:], AF.Tanh)
                        hf = sp.tile([128, CH, B], DT_F32, tag="hfD")
                        nc.vector.tensor_mul(hf[:], so[:], th[:])
                        nc.vector.tensor_copy(
                            outH[:, :, bass.ds(iv0 + i, 1), :], hf[:])
                        if outC is not None:
                            nc.vector.tensor_copy(
                                outC[:, :, bass.ds(iv0 + i, 1), :], cf[:])
                        ms = msk[:, :, i * B:(i + 1) * B]
                        if not is_word:
                            nc.vector.tensor_mul(hnext[:], hf[:], ms)
                            nc.vector.tensor_mul(cnext[:], cf[:], ms)
                        else:
                            d1 = sp.tile([128, CH, B], DT_F32, tag="d1D")
                            nc.vector.tensor_sub(d1[:], hf[:], hprev[:])
                            nc.vector.tensor_mul(d1[:], d1[:], ms)
                            nc.vector.tensor_add(hnext[:], hprev[:], d1[:])
                            d2 = sp.tile([128, CH, B], DT_F32, tag="d2D")
                            nc.vector.tensor_sub(d2[:], cf[:], cprev[:])
                            nc.vector.tensor_mul(d2[:], d2[:], ms)
                            nc.vector.tensor_add(cnext[:], cprev[:], d2[:])

                tc.For_i_unrolled_general(0, S, 1, blk, max_unroll=unroll)

        # Phase D: subword chain  + Phase E: WI GEMM
        if "D" in phases:
         with tc.tile_pool(name="histB", bufs=1) as histB:
            h1T = histB.tile([128, CH, S, B], DT_BF, tag="h1T")
            c1T = histB.tile([128, CH, S, B], DT_BF, tag="c1T")
            dec_chain(swhh, SDT, h1T, c1T, keep6, is_word=False)

            if "E" in phases:
             with tc.tile_pool(name="wE", bufs=1) as wp, \
                 tc.tile_pool(name="gE", bufs=3) as pool, \
                 tc.tile_pool(name="gE_ps", bufs=2,
                              space=bass.MemorySpace.PSUM) as psp:
                ones = wp.tile([1, NB], DT_BF, tag="onesE")
                nc.vector.memset(ones[:], 1.0)
                wwih_sb = load_w(wp, wwih, "wwih_sb")
                wb_sb = load_w(wp, wbias, "wbias_sb")
                for m in range(GM):
                    for nb in range(NBLK):
                        ps = psp.tile([128, NB], DT_F32, tag="psE")
                        t0 = (nb * NB) // B
                        rows = NB // B
                        for k in range(C2):
                            src = h1T if k < CH else c1T
                            kk = k if k < CH else k - CH
                            fl = src[:, kk].rearrange("p a b -> p (a b)")
                            nc.tensor.matmul(
                                ps[:], wwih_sb[:, k, bass.ts(m, 128)],
                                fl[:, t0 * B:t0 * B + NB],
                                start=(k == 0), stop=False)
                        nc.tensor.matmul(
                            ps[:], wb_sb[:, bass.ts(m, 128)], ones[:],
                            start=False, stop=True)
                        ot = pool.tile([128, NB], DT_BF, tag="goutE")
                        nc.vector.tensor_copy(ot[:], ps[:])
                        dma_eng().dma_start(WIT[:, m, bass.ts(nb, NB)], ot[:])

        # Phase F: word chain  + Phase G: wh1 classifier
        if "F" in phases:
         with tc.tile_pool(name="histC", bufs=1) as histC:
            wh1T = histC.tile([128, CH, S, B], DT_BF, tag="wh1T")
            dec_chain(wwhh, WIT, wh1T, None, wsel6, is_word=True)

            if "G" in phases:
             with tc.tile_pool(name="wG", bufs=1) as wp, \
                 tc.tile_pool(name="gG", bufs=3) as pool, \
                 tc.tile_pool(name="gG_ps", bufs=2,
                              space=bass.MemorySpace.PSUM) as psp:
                c1_sb = load_w(wp, cls1T, "c1_sb")
                for nb in range(NBLK):
                    ps = psp.tile([2, NB], DT_F32, tag="psG")
                    t0 = (nb * NB) // B
                    rows = NB // B
                    for k in range(CH):
                        fl = wh1T[:, k].rearrange("p a b -> p (a b)")
                        nc.tensor.matmul(
                            ps[:], c1_sb[:, k, :],
                            fl[:, t0 * B:t0 * B + NB],
                            start=(k == 0), stop=(k == CH - 1))
                    ot = pool.tile([2, NB], DT_F32, tag="goutG")
                    nc.vector.tensor_copy(ot[:], ps[:])
                    nc.sync.dma_start(wcls_t[:, bass.ts(nb, NB)], ot[:])

    nc.compile()
    return nc


# --------------------------------------------------------------------------
# host-side preparation / assembly
# --------------------------------------------------------------------------

def _wT_tiles(w, KD):
    """weight [M, K] fp32 -> W.T as [128, K/128, M] bf16."""
    M, K = w.shape
    assert K == KD
    wt = np.ascontiguousarray(w.T).reshape(K // 128, 128, M)
    return np.ascontiguousarray(wt.transpose(1, 0, 2)).astype(BF16)


def _mask6(mask_tb, CH):
    """mask [S, B] -> [128, CH, S*B] broadcast layout (bf16)."""
    S_, B_ = mask_tb.shape
    flat = mask_tb.reshape(-1)
    out = np.broadcast_to(flat[None, None, :], (128, CH, S_ * B_))
    return np.ascontiguousarray(out).astype(BF16)


def prepare_inputs(inputs, S, B, H, ncores):
    CH = H // 128
    x = np.asarray(inputs["hidden_state"], np.float32)
    golds = np.asarray(inputs["golds"]).astype(np.int32)
    assert x.shape[0] == ncores * B

    shared = dict(
        wih_f=_wT_tiles(np.asarray(inputs["lstm_Wih_f"], np.float32), H),
        whh_f=_wT_tiles(np.asarray(inputs["lstm_Whh_f"], np.float32), H),
        wih_b=_wT_tiles(np.asarray(inputs["lstm_Wih_b"], np.float32), H),
        whh_b=_wT_tiles(np.asarray(inputs["lstm_Whh_b"], np.float32), H),
        swih=_wT_tiles(np.asarray(inputs["subw_Wih"], np.float32), 2 * H),
        swhh=_wT_tiles(np.asarray(inputs["subw_Whh"], np.float32), H),
        wwih=_wT_tiles(np.asarray(inputs["word_Wih"], np.float32), 2 * H),
        wwhh=_wT_tiles(np.asarray(inputs["word_Whh"], np.float32), H),
        cls1T=_wT_tiles(np.asarray(inputs["cls_W"], np.float32)[:, :H], H),
        cls2T=_wT_tiles(np.asarray(inputs["cls_W"], np.float32)[:, H:], 2 * H),
        bias_f=np.asarray(inputs["lstm_b_f"], np.float32)[None, :].astype(BF16),
        bias_b=np.asarray(inputs["lstm_b_b"], np.float32)[None, :].astype(BF16),
        sbias=np.asarray(inputs["subw_b"], np.float32)[None, :].astype(BF16),
        wbias=np.asarray(inputs["word_b"], np.float32)[None, :].astype(BF16),
    )

    in_maps = []
    for c in range(ncores):
        xs = x[c * B:(c + 1) * B]                      # [B, S, H]
        xt = xs.transpose(2, 1, 0).reshape(CH, 128, S, B)
        xT = np.ascontiguousarray(
            xt.transpose(1, 0, 2, 3).reshape(128, CH, S * B)).astype(BF16)
        xTr = np.ascontiguousarray(
            xt[:, :, ::-1, :].transpose(1, 0, 2, 3).reshape(
                128, CH, S * B)).astype(BF16)
        g = golds[c * B:(c + 1) * B, 1:]               # [B, S-1]
        m = (g > 0).astype(np.float32).T               # [S-1, B]
        pad = np.zeros((1, B), np.float32)
        keep_p = np.concatenate([1.0 - m, pad], 0)     # [S, B]
        sel_p = np.concatenate([m, pad], 0)
        im = dict(shared)
        im.update(xT=xT, xTr=xTr,
                  keep6=_mask6(keep_p, CH), wsel6=_mask6(sel_p, CH))
        in_maps.append(im)

    assembly = dict(cls_b=np.asarray(inputs["cls_b"], np.float32),
                    S=S, B=B, ncores=ncores)
    return in_maps, assembly


def assemble_output(results, assembly):
    S, B, ncores = assembly["S"], assembly["B"], assembly["ncores"]
    cls_b = assembly["cls_b"]
    out = np.empty((ncores * B, S, 2), np.float32)
    for c in range(ncores):
        cx = results[c]["cx_t"].reshape(2, S, B)
        wc = results[c]["wcls_t"].reshape(2, S, B)
        for j in range(2):
            # out[:, t] (t>=1) = cx[:, t] + wcls[:, t-1] + cls_b
            out[c * B:(c + 1) * B, 1:, j] = (
                cx[j, 1:, :] + wc[j, :S - 1, :]).T + cls_b[j]
    out[:, 0, 0] = -1.0
    out[:, 0, 1] = 1.0
    return out


# --------------------------------------------------------------------------
# entry point
# --------------------------------------------------------------------------

_CACHE = {}


def _get_program():
    if "full" not in _CACHE:
        _CACHE["full"] = build_program(FULL["S"], FULL["B"], FULL["H"],
                                       num_devices=FULL["NCORES"])
    return _CACHE["full"]


def run(inputs, trace=False):
    nc = _get_program()
    in_maps, assembly = prepare_inputs(
        inputs, FULL["S"], FULL["B"], FULL["H"], FULL["NCORES"])
    res = run_bass_kernel_spmd(
        nc, in_maps, core_ids=list(range(FULL["NCORES"])), trace=trace)
    out = assemble_output(res.results, assembly)
    return out, res


def kernel(**inputs) -> np.ndarray:
    out, _ = run(inputs, trace=False)
    return out
